# revision 5
# baseline (speedup 1.0000x reference)
"""EdgeOnlyConv GNN message-passing kernel for Trainium2 (8 NeuronCores).

out[e] = concat(x[src[e]], x[dest[e]], edge_attr[e]) @ W.T + b

Strategy (edge-parallel across 8 cores, x & weights replicated):
  Phase A (per core): YC = x @ [W_src.T | W_dest.T] + [b | 0]  -> DRAM [N_pad, 256]
  Phase B (per core): for each 2048-edge supertile:
    - indirect-DMA gather YC[src][:128] and YC[dest][128:] (multi-row per call)
    - z = edge_attr @ W_edge.T via PE matmuls (edge_attr passed host-transposed)
    - out = gather_src + gather_dest + z (DVE adds), batched store
"""

import sys
import numpy as np

if "/opt/trn_rl_repo" not in sys.path:
    sys.path.insert(0, "/opt/trn_rl_repo")

P = 128

# Full-problem config (hardcoded per contest rules).
N_CORES = 8
N_NODES = 50000
N_IN_NODE = 128
N_IN_EDGE = 64
N_OUT = 128
N_EDGES = 1000000
E_CORE = N_EDGES // N_CORES          # 125000
K_SUP = 16                           # 128-edge tiles per supertile
T_TILES = ((E_CORE + P - 1) // P + K_SUP - 1) // K_SUP * K_SUP   # 992
E_PAD = T_TILES * P                  # 126976
S_SUP = T_TILES // K_SUP             # 62
NODES_PAD = (N_NODES + P - 1) // P * P   # 50048
A_TILES = NODES_PAD // P             # 391

TABLE_DT_NP = np.float32             # dtype of the YC node table (gather payload)


def build_program(
    n_cores=N_CORES,
    nodes_pad=NODES_PAD,
    e_pad=E_PAD,
    k_sup=K_SUP,
    table_dt_np=TABLE_DT_NP,
):
    """Build the Bass program. Returns the compiled Bacc object."""
    import concourse.mybir as mybir
    import concourse.tile as tile
    from concourse import bacc
    from concourse import bass as cbass
    from concourse.masks import make_identity

    f32 = mybir.dt.float32
    i32 = mybir.dt.int32
    table_dt = mybir.dt.from_np(np.dtype(table_dt_np))

    a_tiles = nodes_pad // P
    t_tiles = e_pad // P
    s_sup = t_tiles // k_sup
    d_comb = 2 * N_OUT                      # 256: [src-half | dest-half]

    nc = bacc.Bacc("TRN2", target_bir_lowering=False, debug=False,
                   num_devices=n_cores)

    x_d = nc.dram_tensor("x", [nodes_pad, N_IN_NODE], f32, kind="ExternalInput").ap()
    wct_d = nc.dram_tensor("wct", [N_IN_NODE, d_comb], f32, kind="ExternalInput").ap()
    wet_d = nc.dram_tensor("wet", [N_IN_EDGE, N_OUT], f32, kind="ExternalInput").ap()
    bias_d = nc.dram_tensor("bias", [P, d_comb], f32, kind="ExternalInput").ap()
    isrc_d = nc.dram_tensor("isrc", [P, t_tiles], i32, kind="ExternalInput").ap()
    idst_d = nc.dram_tensor("idst", [P, t_tiles], i32, kind="ExternalInput").ap()
    eat_d = nc.dram_tensor("eat", [N_IN_EDGE, e_pad], f32, kind="ExternalInput").ap()
    out_d = nc.dram_tensor("out", [e_pad, N_OUT], f32, kind="ExternalOutput").ap()
    # separate src/dest node tables: indirect-DMA HW reads one full table row
    # (= coef elements) per index, so the gather chunk must equal the row width
    ys_d = nc.dram_tensor("ys", [nodes_pad, N_OUT], table_dt, kind="Internal").ap()
    yd_d = nc.dram_tensor("yd", [nodes_pad, N_OUT], table_dt, kind="Internal").ap()

    with tile.TileContext(nc) as tc:
        with tc.tile_pool(name="static", bufs=1) as spool:
            ident = spool.tile([P, P], f32)
            make_identity(nc, ident[:])
            wct_sb = spool.tile([N_IN_NODE, d_comb], f32)
            nc.sync.dma_start(wct_sb[:], wct_d[:, :])
            wet_sb = spool.tile([N_IN_EDGE, N_OUT], f32)
            nc.sync.dma_start(wet_sb[:], wet_d[:, :])
            bias_sb = spool.tile([P, d_comb], f32)
            nc.sync.dma_start(bias_sb[:], bias_d[:, :])
            isrc_sb = spool.tile([P, t_tiles], i32)
            nc.sync.dma_start(isrc_sb[:], isrc_d[:, :])
            idst_sb = spool.tile([P, t_tiles], i32)
            nc.sync.dma_start(idst_sb[:], idst_d[:, :])

            # ---- Phase A: YC = x @ WcT + bias ----
            with tc.tile_pool(name="a_sbuf", bufs=3) as apool, \
                 tc.tile_pool(name="a_ps_xt", bufs=2, space="PSUM") as aps_xt, \
                 tc.tile_pool(name="a_ps_yc", bufs=2, space="PSUM") as aps_yc:
                for g0 in range(0, a_tiles, 4):
                    gn = min(4, a_tiles - g0)
                    xt_ps = aps_xt.tile([P, 4 * P], f32)
                    for i in range(gn):
                        x_sb = apool.tile([P, N_IN_NODE], f32, tag="x_sb")
                        nc.sync.dma_start(
                            x_sb[:], x_d[(g0 + i) * P:(g0 + i + 1) * P, :])
                        nc.tensor.transpose(
                            xt_ps[:, i * P:(i + 1) * P], x_sb[:], ident[:])
                    xt_sb = apool.tile([P, 4 * P], f32, tag="xt_sb")
                    nc.vector.tensor_copy(xt_sb[:, :gn * P], xt_ps[:, :gn * P])
                    # two node-tiles share one yc PSUM bank ([128, 512])
                    for h0 in range(0, gn, 2):
                        hn = min(2, gn - h0)
                        yc_ps = aps_yc.tile([P, 2 * d_comb], f32, tag="yc_ps")
                        for i in range(h0, h0 + hn):
                            nc.tensor.matmul(
                                yc_ps[:, (i - h0) * d_comb:(i - h0 + 1) * d_comb],
                                lhsT=xt_sb[:, i * P:(i + 1) * P],
                                rhs=wct_sb[:],
                                start=True, stop=True)
                        yc_sb = apool.tile([P, 2 * d_comb], table_dt, tag="yc_sb")
                        for i in range(h0, h0 + hn):
                            nc.vector.tensor_add(
                                yc_sb[:, (i - h0) * d_comb:(i - h0 + 1) * d_comb],
                                yc_ps[:, (i - h0) * d_comb:(i - h0 + 1) * d_comb],
                                bias_sb[:])
                        for i in range(h0, h0 + hn):
                            rows = slice((g0 + i) * P, (g0 + i + 1) * P)
                            c0 = (i - h0) * d_comb
                            nc.sync.dma_start(
                                ys_d[rows, :], yc_sb[:, c0:c0 + N_OUT])
                            nc.sync.dma_start(
                                yd_d[rows, :], yc_sb[:, c0 + N_OUT:c0 + d_comb])

            # ensure all YC writes land before any gather
            tc.strict_bb_all_engine_barrier()

            # ---- Phase B: per-supertile edge processing ----
            out_v = out_d.rearrange("(t p) o -> p t o", p=P)
            sup_cols = k_sup * P
            with tc.tile_pool(name="b_sbuf", bufs=3) as bpool, \
                 tc.tile_pool(name="b_psum", bufs=4, space="PSUM") as bpsum:
                for s in range(s_sup):
                    j0 = s * k_sup
                    # indirect-DMA HW consumes ONE index per partition per
                    # call (reads out-free-size contiguous elements from that
                    # row), so gathers are per-128-edge-tile
                    ysrc = bpool.tile([P, sup_cols], table_dt, tag="ysrc")
                    for j in range(k_sup):
                        nc.gpsimd.indirect_dma_start(
                            out=ysrc[:, j * P:(j + 1) * P], out_offset=None,
                            in_=ys_d[:, :],
                            in_offset=cbass.IndirectOffsetOnAxis(
                                ap=isrc_sb[:, j0 + j:j0 + j + 1], axis=0))
                    ydst = bpool.tile([P, sup_cols], table_dt, tag="ydst")
                    for j in range(k_sup):
                        nc.gpsimd.indirect_dma_start(
                            out=ydst[:, j * P:(j + 1) * P], out_offset=None,
                            in_=yd_d[:, :],
                            in_offset=cbass.IndirectOffsetOnAxis(
                                ap=idst_sb[:, j0 + j:j0 + j + 1], axis=0))
                    eat_sb = bpool.tile([N_IN_EDGE, sup_cols], f32, tag="eat_sb")
                    nc.sync.dma_start(
                        eat_sb[:], eat_d[:, j0 * P:(j0 + k_sup) * P])
                    tsum = bpool.tile([P, sup_cols], f32, tag="tsum")
                    nc.vector.tensor_add(tsum[:], ysrc[:], ydst[:])
                    outsb = bpool.tile([P, sup_cols], f32, tag="outsb")
                    for bank in range(k_sup // 4):
                        z_ps = bpsum.tile([P, 4 * P], f32, tag="z_ps")
                        for jj in range(4):
                            t_loc = bank * 4 + jj
                            nc.tensor.matmul(
                                z_ps[:, jj * P:(jj + 1) * P],
                                lhsT=eat_sb[:, t_loc * P:(t_loc + 1) * P],
                                rhs=wet_sb[:],
                                start=True, stop=True)
                        nc.vector.tensor_add(
                            outsb[:, bank * 4 * P:(bank + 1) * 4 * P],
                            z_ps[:],
                            tsum[:, bank * 4 * P:(bank + 1) * 4 * P])
                    nc.sync.dma_start(out_v[:, j0:j0 + k_sup, :], outsb[:])

    nc.compile()
    return nc


def prep_inputs(x, edge_index, edge_attr, W, b,
                n_cores=N_CORES, e_pad=E_PAD, nodes_pad=NODES_PAD,
                table_dt_np=TABLE_DT_NP):
    """Host-side input prep: shard + pad + layout. Returns list of in_maps."""
    x = np.asarray(x, dtype=np.float32)
    edge_index = np.asarray(edge_index)
    edge_attr = np.asarray(edge_attr, dtype=np.float32)
    W = np.asarray(W, dtype=np.float32)
    b = np.asarray(b, dtype=np.float32)

    n_nodes, d_node = x.shape
    e_total = edge_index.shape[1]
    e_core = e_total // n_cores
    d_out = W.shape[0]
    d_edge = edge_attr.shape[1]
    t_tiles = e_pad // P

    x_pad = np.zeros((nodes_pad, d_node), dtype=np.float32)
    x_pad[:n_nodes] = x
    wct = np.ascontiguousarray(
        np.concatenate([W[:, :d_node].T, W[:, d_node:2 * d_node].T], axis=1))
    wet = np.ascontiguousarray(W[:, 2 * d_node:].T)
    bias_full = np.ascontiguousarray(
        np.concatenate([np.tile(b, (P, 1)),
                        np.zeros((P, d_out), dtype=np.float32)], axis=1))

    src = np.ascontiguousarray(edge_index[0]).astype(np.int32)
    dst = np.ascontiguousarray(edge_index[1]).astype(np.int32)

    in_maps = []
    for c in range(n_cores):
        lo, hi = c * e_core, (c + 1) * e_core
        src_pad = np.zeros(e_pad, dtype=np.int32)
        src_pad[:e_core] = src[lo:hi]
        dst_pad = np.zeros(e_pad, dtype=np.int32)
        dst_pad[:e_core] = dst[lo:hi]
        isrc = np.ascontiguousarray(src_pad.reshape(t_tiles, P).T)
        idst = np.ascontiguousarray(dst_pad.reshape(t_tiles, P).T)
        ea_pad = np.zeros((e_pad, d_edge), dtype=np.float32)
        ea_pad[:e_core] = edge_attr[lo:hi]
        eat = np.ascontiguousarray(ea_pad.T)
        in_maps.append({
            "x": x_pad, "wct": wct, "wet": wet, "bias": bias_full,
            "isrc": isrc, "idst": idst, "eat": eat,
        })
    return in_maps


_NC_CACHE = {}


def _get_program():
    key = "full"
    if key not in _NC_CACHE:
        _NC_CACHE[key] = build_program()
    return _NC_CACHE[key]


def run_on_hw(in_maps, nc=None, trace=False, n_cores=N_CORES):
    from concourse import bass_utils
    if nc is None:
        nc = _get_program()
    kw = {}
    if trace:
        _install_profile_hook(bass_utils)
        kw["trace"] = True
    res = bass_utils.run_bass_kernel_spmd(
        nc, in_maps, core_ids=list(range(n_cores)), **kw)
    return res


def _install_profile_hook(bass_utils):
    """Inject the NTFF profile hook missing from this image's antenv."""
    import types
    if "antenv.axon_hooks" in sys.modules:
        return
    try:
        from trn_agent_boot.trn_boot import _ntff_profile_via_ctypes
        hook = _ntff_profile_via_ctypes("/opt/axon/libaxon_pjrt.so")
    except Exception:
        hook = None
    mod = types.ModuleType("antenv.axon_hooks")
    mod.get_axon_ntff_profile_hook = lambda: hook
    mod.set_axon_ntff_profile_hook = lambda h: None
    sys.modules["antenv.axon_hooks"] = mod
    bass_utils.upload_artifacts = lambda tmpdir: f"file://{tmpdir}"


def kernel(x, edge_index, edge_attr, W, b):
    in_maps = prep_inputs(x, edge_index, edge_attr, W, b)
    res = run_on_hw(in_maps)
    e_core = edge_index.shape[1] // N_CORES
    outs = [res.results[c]["out"][:e_core] for c in range(N_CORES)]
    return np.concatenate(outs, axis=0)


# revision 14
# speedup vs baseline: 1.3677x; 1.3677x over previous
"""EdgeOnlyConv GNN message-passing kernel for Trainium2 (8 NeuronCores).

out[e] = concat(x[src[e]], x[dest[e]], edge_attr[e]) @ W.T + b

Strategy (edge-parallel across 8 cores, x & weights replicated):
  Phase A (per core): node tables Ys = x @ W_src.T + b, Yd = x @ W_dest.T,
    stored fp16 as PAIR-ROW tables [N/2, 256] (row k = nodes 2k,2k+1).
  Phase B (per core), per 2048-edge supertile:
    - one dma_gather per endpoint table (int16 pair indices = node>>1,
      2048 idx/call) fetches both nodes of each pair (512B rows)
    - DVE parity select picks the right half per edge (host parity masks)
    - z = edge_attr @ W_edge.T on PE (edge_attr passed host-transposed)
    - out = sel_src + sel_dst + z, batched store
"""

import sys
import numpy as np

if "/opt/trn_rl_repo" not in sys.path:
    sys.path.insert(0, "/opt/trn_rl_repo")

P = 128
CHUNK_IDX = 512   # indices per dma_gather call (HW descriptor-ring limit)

N_CORES = 8
N_NODES = 50000
N_IN_NODE = 128
N_IN_EDGE = 64
N_OUT = 128
N_EDGES = 1000000
E_CORE = N_EDGES // N_CORES          # 125000
K_SUP = 16                           # 128-edge tiles per supertile
T_TILES = ((E_CORE + P - 1) // P + K_SUP - 1) // K_SUP * K_SUP   # 992
E_PAD = T_TILES * P                  # 126976
S_SUP = T_TILES // K_SUP             # 62
NODES_PAD = (N_NODES + 255) // 256 * 256   # 50176 (pair rows: 25088)
A_TILES = NODES_PAD // P             # 392


def build_program(
    n_cores=N_CORES,
    nodes_pad=NODES_PAD,
    e_pad=E_PAD,
    k_sup=K_SUP,
):
    """Build the Bass program. Returns the compiled Bacc object."""
    import concourse.mybir as mybir
    import concourse.tile as tile
    from concourse import bacc
    from concourse import bass as cbass

    f32 = mybir.dt.float32
    f16 = mybir.dt.float16
    i16 = mybir.dt.int16

    a_tiles = nodes_pad // P
    t_tiles = e_pad // P
    s_sup = t_tiles // k_sup
    n_idx = k_sup * P                       # indices per dma_gather call
    idx_cols = n_idx // 16                  # int16 idx columns per supertile
    d_comb = 2 * N_OUT                      # 256
    pair_rows = nodes_pad // 2

    nc = bacc.Bacc("TRN2", target_bir_lowering=False, debug=False,
                   num_devices=n_cores)

    x_d = nc.dram_tensor("x", [nodes_pad, N_IN_NODE], f16, kind="ExternalInput").ap()
    wct_d = nc.dram_tensor("wct", [N_IN_NODE, d_comb], f16, kind="ExternalInput").ap()
    wet_d = nc.dram_tensor("wet", [N_IN_EDGE, N_OUT], f32, kind="ExternalInput").ap()
    bias_d = nc.dram_tensor("bias", [P, 2 * d_comb], f32, kind="ExternalInput").ap()
    gs_d = nc.dram_tensor("gs", [P, s_sup * idx_cols], i16, kind="ExternalInput").ap()
    gd_d = nc.dram_tensor("gd", [P, s_sup * idx_cols], i16, kind="ExternalInput").ap()
    ps_d = nc.dram_tensor("ps", [P, 2 * t_tiles], f16, kind="ExternalInput").ap()
    pd_d = nc.dram_tensor("pd", [P, 2 * t_tiles], f16, kind="ExternalInput").ap()
    eat_d = nc.dram_tensor("eat", [N_IN_EDGE, e_pad], f32, kind="ExternalInput").ap()
    out_d = nc.dram_tensor("out", [e_pad, N_OUT], f32, kind="ExternalOutput").ap()
    ys_d = nc.dram_tensor("ys", [pair_rows, d_comb], f16, kind="Internal").ap()
    yd_d = nc.dram_tensor("yd", [pair_rows, d_comb], f16, kind="Internal").ap()
    # node-row views of the pair tables for phase A stores
    ys_v = ys_d.rearrange("k (j f) -> (k j) f", j=2)
    yd_v = yd_d.rearrange("k (j f) -> (k j) f", j=2)

    GRP = 8  # node tiles per phase-A group

    with tile.TileContext(nc) as tc:
        with tc.tile_pool(name="static", bufs=1) as spool:
            wct_sb = spool.tile([N_IN_NODE, d_comb], f16)
            nc.sync.dma_start(wct_sb[:], wct_d[:, :])
            wet_sb = spool.tile([N_IN_EDGE, N_OUT], f32)
            nc.sync.dma_start(wet_sb[:], wet_d[:, :])
            bias_sb = spool.tile([P, 2 * d_comb], f32)
            nc.sync.dma_start(bias_sb[:], bias_d[:, :])
            gs_sb = spool.tile([P, s_sup * idx_cols], i16)
            nc.sync.dma_start(gs_sb[:], gs_d[:, :])
            gd_sb = spool.tile([P, s_sup * idx_cols], i16)
            nc.sync.dma_start(gd_sb[:], gd_d[:, :])
            ps_sb = spool.tile([P, 2 * t_tiles], f16)
            nc.sync.dma_start(ps_sb[:], ps_d[:, :])
            pd_sb = spool.tile([P, 2 * t_tiles], f16)
            nc.sync.dma_start(pd_sb[:], pd_d[:, :])

            # ---- Phase A: Ys = x @ Wsrc.T + b, Yd = x @ Wdest.T (fp16) ----
            with tc.tile_pool(name="a_sbuf", bufs=3) as apool, \
                 tc.tile_pool(name="a_ps_yc", bufs=4, space="PSUM") as aps_yc:
                for g0 in range(0, a_tiles, GRP):
                    gn = min(GRP, a_tiles - g0)
                    xt_sb = apool.tile([P, GRP * P], f16, tag="xt_sb")
                    nc.sync.dma_start(
                        xt_sb[:, :gn * P],
                        x_d[g0 * P:(g0 + gn) * P, :], transpose=True)
                    yc_sb = apool.tile([P, GRP * d_comb], f16, tag="yc_sb")
                    for h0 in range(0, gn, 2):
                        hn = min(2, gn - h0)
                        yc_ps = aps_yc.tile([P, 2 * d_comb], f32, tag="yc_ps")
                        for i in range(h0, h0 + hn):
                            nc.tensor.matmul(
                                yc_ps[:, (i - h0) * d_comb:(i - h0 + 1) * d_comb],
                                lhsT=xt_sb[:, i * P:(i + 1) * P],
                                rhs=wct_sb[:], start=True, stop=True)
                        nc.vector.tensor_add(
                            yc_sb[:, h0 * d_comb:(h0 + hn) * d_comb],
                            yc_ps[:, :hn * d_comb],
                            bias_sb[:, :hn * d_comb])
                    # batched stores: ys rows g0*P..(g0+gn)*P from strided cols
                    yc_v = yc_sb.rearrange("p (g c) -> p g c", c=d_comb)
                    ys_rows = ys_v[g0 * P:(g0 + gn) * P, :].rearrange(
                        "(g p) f -> p g f", p=P)
                    yd_rows = yd_v[g0 * P:(g0 + gn) * P, :].rearrange(
                        "(g p) f -> p g f", p=P)
                    nc.sync.dma_start(ys_rows[:, :, :], yc_v[:, :gn, 0:N_OUT])
                    nc.sync.dma_start(yd_rows[:, :, :], yc_v[:, :gn, N_OUT:d_comb])

            tc.strict_bb_all_engine_barrier()

            # ---- Phase B ----
            out_v = out_d.rearrange("(t p) o -> p t o", p=P)
            sup_cols = k_sup * P
            with tc.tile_pool(name="b_sbuf", bufs=2) as bpool, \
                 tc.tile_pool(name="b_psum", bufs=4, space="PSUM") as bpsum:
                for s in range(s_sup):
                    j0 = s * k_sup
                    # 512-idx chunks: larger single dma_gather calls overflow
                    # the SWDGE descriptor ring and hang the device
                    ch_idx = min(CHUNK_IDX, n_idx)
                    ch_tiles = ch_idx // P
                    ch_cols = ch_idx // 16
                    n_ch = n_idx // ch_idx
                    gsrc = bpool.tile([P, k_sup, d_comb], f16, tag="gsrc")
                    gdst = bpool.tile([P, k_sup, d_comb], f16, tag="gdst")
                    for c in range(n_ch):
                        c0 = s * idx_cols + c * ch_cols
                        nc.gpsimd.dma_gather(
                            out_ap=gsrc[:, c * ch_tiles:(c + 1) * ch_tiles, :],
                            in_ap=ys_d[:, :],
                            idxs_ap=gs_sb[:, c0:c0 + ch_cols],
                            num_idxs=ch_idx, num_idxs_reg=ch_idx,
                            elem_size=d_comb)
                        nc.gpsimd.dma_gather(
                            out_ap=gdst[:, c * ch_tiles:(c + 1) * ch_tiles, :],
                            in_ap=yd_d[:, :],
                            idxs_ap=gd_sb[:, c0:c0 + ch_cols],
                            num_idxs=ch_idx, num_idxs_reg=ch_idx,
                            elem_size=d_comb)
                    eat_sb = bpool.tile([N_IN_EDGE, sup_cols], f32, tag="eat_sb")
                    nc.sync.dma_start(
                        eat_sb[:], eat_d[:, j0 * P:(j0 + k_sup) * P])

                    # parity select: res = lo + par*(hi-lo), per endpoint
                    par_s = ps_sb[:, 2 * j0:2 * (j0 + k_sup)].rearrange(
                        "p (g two) -> p g two", two=2)
                    par_d = pd_sb[:, 2 * j0:2 * (j0 + k_sup)].rearrange(
                        "p (g two) -> p g two", two=2)
                    us = bpool.tile([P, k_sup, N_OUT], f16, tag="us")
                    nc.vector.tensor_sub(
                        us[:, :, :], gsrc[:, :, N_OUT:d_comb], gsrc[:, :, 0:N_OUT])
                    nc.vector.tensor_mul(
                        us[:, :, :], us[:, :, :],
                        par_s[:, :, 0:1].to_broadcast([P, k_sup, N_OUT]))
                    ud = bpool.tile([P, k_sup, N_OUT], f16, tag="ud")
                    nc.vector.tensor_sub(
                        ud[:, :, :], gdst[:, :, N_OUT:d_comb], gdst[:, :, 0:N_OUT])
                    nc.vector.tensor_mul(
                        ud[:, :, :], ud[:, :, :],
                        par_d[:, :, 0:1].to_broadcast([P, k_sup, N_OUT]))
                    q = bpool.tile([P, k_sup, N_OUT], f32, tag="q")
                    nc.vector.tensor_add(
                        q[:, :, :], gsrc[:, :, 0:N_OUT], gdst[:, :, 0:N_OUT])
                    tsum = bpool.tile([P, k_sup, N_OUT], f32, tag="tsum")
                    nc.vector.tensor_add(tsum[:, :, :], us[:, :, :], ud[:, :, :])
                    nc.vector.tensor_add(tsum[:, :, :], tsum[:, :, :], q[:, :, :])

                    outsb = bpool.tile([P, sup_cols], f32, tag="outsb")
                    tsum_f = tsum.rearrange("p g o -> p (g o)")
                    for bank in range(k_sup // 4):
                        z_ps = bpsum.tile([P, 4 * P], f32, tag="z_ps")
                        for jj in range(4):
                            t_loc = bank * 4 + jj
                            nc.tensor.matmul(
                                z_ps[:, jj * P:(jj + 1) * P],
                                lhsT=eat_sb[:, t_loc * P:(t_loc + 1) * P],
                                rhs=wet_sb[:], start=True, stop=True)
                        nc.vector.tensor_add(
                            outsb[:, bank * 4 * P:(bank + 1) * 4 * P],
                            z_ps[:], tsum_f[:, bank * 4 * P:(bank + 1) * 4 * P])
                    nc.sync.dma_start(out_v[:, j0:j0 + k_sup, :], outsb[:])

    nc.compile()
    return nc


def _idx_wrap16(seq_i16, n_idx):
    """Pack a flat int16 index sequence into the dma_gather SBUF layout:
    index i at (partition i%16, column i//16), replicated to 8x16 rows."""
    cols = n_idx // 16
    blocks = seq_i16.reshape(-1, cols, 16)           # [S, cols, 16]
    arr = blocks.transpose(0, 2, 1).reshape(-1, 16, cols)  # [S, 16, cols]
    out = np.concatenate([np.tile(a, (8, 1)) for a in arr], axis=1)
    return np.ascontiguousarray(out)                 # [128, S*cols]


def prep_inputs(x, edge_index, edge_attr, W, b,
                n_cores=N_CORES, e_pad=E_PAD, nodes_pad=NODES_PAD,
                k_sup=K_SUP):
    """Host-side input prep: shard + pad + layout. Returns list of in_maps."""
    x = np.asarray(x, dtype=np.float32)
    edge_index = np.asarray(edge_index)
    edge_attr = np.asarray(edge_attr, dtype=np.float32)
    W = np.asarray(W, dtype=np.float32)
    b = np.asarray(b, dtype=np.float32)

    n_nodes, d_node = x.shape
    e_total = edge_index.shape[1]
    e_core = e_total // n_cores
    d_out = W.shape[0]
    d_edge = edge_attr.shape[1]
    t_tiles = e_pad // P
    n_idx = k_sup * P

    x_pad = np.zeros((nodes_pad, d_node), dtype=np.float16)
    x_pad[:n_nodes] = x.astype(np.float16)
    wct = np.ascontiguousarray(np.concatenate(
        [W[:, :d_node].T, W[:, d_node:2 * d_node].T], axis=1)).astype(np.float16)
    wet = np.ascontiguousarray(W[:, 2 * d_node:].T)
    bias_comb = np.concatenate(
        [np.tile(b, (P, 1)), np.zeros((P, d_out), dtype=np.float32)], axis=1)
    bias_full = np.ascontiguousarray(
        np.tile(bias_comb, (1, 2)).astype(np.float32))

    src = np.ascontiguousarray(edge_index[0]).astype(np.int32)
    dst = np.ascontiguousarray(edge_index[1]).astype(np.int32)

    in_maps = []
    for c in range(n_cores):
        lo, hi = c * e_core, (c + 1) * e_core
        src_pad = np.zeros(e_pad, dtype=np.int32)
        src_pad[:e_core] = src[lo:hi]
        dst_pad = np.zeros(e_pad, dtype=np.int32)
        dst_pad[:e_core] = dst[lo:hi]
        chunk = min(CHUNK_IDX, n_idx)
        gs = _idx_wrap16((src_pad >> 1).astype(np.int16), chunk)
        gd = _idx_wrap16((dst_pad >> 1).astype(np.int16), chunk)
        # parity masks in t-major tile layout, duplicated (mask, 0) pairs so
        # device can broadcast-slice [:, :, 0:1]
        ps = np.zeros((P, 2 * t_tiles), dtype=np.float16)
        ps[:, 0::2] = (src_pad & 1).astype(np.float16).reshape(t_tiles, P).T
        pd = np.zeros((P, 2 * t_tiles), dtype=np.float16)
        pd[:, 0::2] = (dst_pad & 1).astype(np.float16).reshape(t_tiles, P).T
        ea_pad = np.zeros((e_pad, d_edge), dtype=np.float32)
        ea_pad[:e_core] = edge_attr[lo:hi]
        eat = np.ascontiguousarray(ea_pad.T)
        in_maps.append({
            "x": x_pad, "wct": wct, "wet": wet, "bias": bias_full,
            "gs": gs, "gd": gd, "ps": ps, "pd": pd, "eat": eat,
        })
    return in_maps


_NC_CACHE = {}


def _get_program():
    key = "full"
    if key not in _NC_CACHE:
        _NC_CACHE[key] = build_program()
    return _NC_CACHE[key]


def run_on_hw(in_maps, nc=None, trace=False, n_cores=N_CORES):
    from concourse import bass_utils
    if nc is None:
        nc = _get_program()
    kw = {}
    if trace:
        _install_profile_hook(bass_utils)
        kw["trace"] = True
    res = bass_utils.run_bass_kernel_spmd(
        nc, in_maps, core_ids=list(range(n_cores)), **kw)
    return res


def _install_profile_hook(bass_utils):
    """Inject the NTFF profile hook missing from this image's antenv."""
    import types
    if "antenv.axon_hooks" in sys.modules:
        return
    try:
        from trn_agent_boot.trn_boot import _ntff_profile_via_ctypes
        hook = _ntff_profile_via_ctypes("/opt/axon/libaxon_pjrt.so")
    except Exception:
        hook = None
    mod = types.ModuleType("antenv.axon_hooks")
    mod.get_axon_ntff_profile_hook = lambda: hook
    mod.set_axon_ntff_profile_hook = lambda h: None
    sys.modules["antenv.axon_hooks"] = mod
    bass_utils.upload_artifacts = lambda tmpdir: f"file://{tmpdir}"


def kernel(x, edge_index, edge_attr, W, b):
    in_maps = prep_inputs(x, edge_index, edge_attr, W, b)
    res = run_on_hw(in_maps)
    e_core = edge_index.shape[1] // N_CORES
    outs = [res.results[c]["out"][:e_core] for c in range(N_CORES)]
    return np.concatenate(outs, axis=0)
